# revision 37
# baseline (speedup 1.0000x reference)
"""AugmentedTripletLoss Trainium2 kernel — 8-core SPMD, row-sharded.

Math (matches reference):
  d2[i,j] = sq_i + sq_j - 2*X@X.T
  ap_i    = sqrt(clip(max_{same class}(d2), 1e-12))
  an_i    = min over (diff-class keys  union  normalized centers) of dist
  loss    = mean(relu(1 + ap - an))

Device strategy (per core, 512 query rows):
  Host sorts rows by class and packs an augmented GEMM so that
  u[q,j] = -2*x_q.x_j + sq_j + BIG*[same class] lands directly in PSUM:
    * data rows (768) as fp8 e4m3, contracted with DoubleRow matmuls
      (two 128-row k-tiles per instruction, ~1.5x bf16 rate),
    * the class/sq block (BIG*onehot + sq hi/lo) as one bf16 k-tile.
  Per-row an = min_j u is one DVE min-reduce per [128,1024] PSUM tile
  (two 512-column key groups share a 2-bank tile; 4 such tiles rotate
  through all 8 PSUM banks). Because rows are class-sorted and each
  core's key columns are rotated by (core*512 - 64), every query tile's
  same-class columns fall in the static window [t*128, t*128+256) of
  the first pair-tile — the ap max-reduce touches one 256-wide slice
  per tile instead of every tile (requires max class size <= 65,
  asserted at prep time).
  Centers (no BIG, csq instead of sq) + padding (u=8192, loses every
  min) ride as a separate 128-column group, min-reduced only.
  Output: per-core [apmax|anmin] [8,128] f32, PE-transposed so the
  store is 8 fat descriptors; host adds sq, does sqrt/relu/mean
  (no device collective). fp8/bf16 split keeps rel err ~1e-4.
"""
import sys

for _p in ("/opt/trn_rl_repo", "/root/.axon_site"):
    if _p not in sys.path:
        sys.path.insert(0, _p)

import numpy as np
import ml_dtypes

import concourse.bass as bass
import concourse.bacc as bacc
import concourse.mybir as mybir
from concourse.tile import TileContext
from concourse.bass_utils import run_bass_kernel_spmd

F32 = mybir.dt.float32
BF16 = mybir.dt.bfloat16
FP8 = mybir.dt.float8e4
ALU = mybir.AluOpType
ACTF = mybir.ActivationFunctionType
AX = mybir.AxisListType
DR = mybir.MatmulPerfMode.DoubleRow

N_CORES = 8
N, D, P = 4096, 768, 100
NQ = N // N_CORES              # 512 query rows per core
MQ = NQ // 128                 # 4 query m-tiles
NG = 3                         # DoubleRow groups (6 fp8 k-tiles of 128)
NJ = 8                         # key column groups of 512
CW = 128                       # center group width (100 centers + 28 pad)
BIG = 16384.0
PAD_U = 8192.0
MARGIN = 1.0
WIN = 64                       # class half-window (max class size must be <=65)

_nc_cache = None


def _build():
    nc = bacc.Bacc("TRN2", target_bir_lowering=False, num_devices=N_CORES)

    kjf_h = nc.declare_dram_parameter("kjf", [NJ * 128, NG * 1024], FP8, isOutput=False)
    kjb_h = nc.declare_dram_parameter("kjb", [NJ * 128, 512], BF16, isOutput=False)
    qqf_h = nc.declare_dram_parameter("qqf", [128, NG * 1024], FP8, isOutput=False)
    qqb_h = nc.declare_dram_parameter("qqb", [128, 512], BF16, isOutput=False)
    kcf_h = nc.declare_dram_parameter("kcf", [128, NG * 256], FP8, isOutput=False)
    kcb_h = nc.declare_dram_parameter("kcb", [128, CW], BF16, isOutput=False)
    out_h = nc.declare_dram_parameter("out", [2 * MQ, 128], F32, isOutput=True)

    with TileContext(nc) as tc:
        from contextlib import ExitStack

        with ExitStack() as ctx:
            const = ctx.enter_context(tc.tile_pool(name="const", bufs=1))
            pmain = ctx.enter_context(tc.tile_pool(name="pmain", bufs=4, space="PSUM"))

            # ---------- loads (two HWDGE rings, issue order = need order) ----------
            qqf = const.tile([128, NG * 1024], FP8)
            kjfs = [
                const.tile([128, NG * 1024], FP8, name=f"kjf{J}") for J in range(NJ)
            ]
            kjbs = [const.tile([128, 512], BF16, name=f"kjb{J}") for J in range(NJ)]

            # qqf arrives in 512-col chunks matching the dd-outer center
            # consumption order; kcf's first 128 cols come as their own tiny
            # transfer so the very first matmul unblocks early.
            for c in range(2 * NG):
                nc.sync.dma_start(
                    out=qqf[:, c * 512 : (c + 1) * 512],
                    in_=qqf_h[:, c * 512 : (c + 1) * 512],
                )
            kcf = const.tile([128, NG * 256], FP8)
            nc.scalar.dma_start(out=kcf[:, 0:128], in_=kcf_h[:, 0:128])
            nc.scalar.dma_start(out=kcf[:, 128:], in_=kcf_h[:, 128:])
            kcb = const.tile([128, CW], BF16)
            nc.scalar.dma_start(out=kcb[:], in_=kcb_h[:, :])
            qqb = const.tile([128, 512], BF16)
            nc.scalar.dma_start(out=qqb[:], in_=qqb_h[:, :])
            for J in range(NJ):
                nc.sync.dma_start(
                    out=kjfs[J][:], in_=kjf_h[J * 128 : (J + 1) * 128, :]
                )
                nc.scalar.dma_start(
                    out=kjbs[J][:], in_=kjb_h[J * 128 : (J + 1) * 128, :]
                )

            from concourse.masks import make_identity
            identf = const.tile([128, 128], F32)
            make_identity(nc, identf[:])

            apan = const.tile([128, 2 * MQ], F32)
            # per-m running-min columns: pairs 0..3 -> 0..3, centers -> 4
            anc = [const.tile([128, 5], F32, name=f"an{m}") for m in range(MQ)]

            def q_lhs(g, m):
                return qqf[:, g * 1024 : (g + 1) * 1024].rearrange(
                    "p (i c) -> p i c", i=2
                )[:, :, m * 128 : (m + 1) * 128]

            # ---------- centers first (dd-outer: consumes qqf chunks in
            # DMA-arrival order, so the PE follows the ramp with no bubbles) --
            pcs = []
            for m in range(MQ):
                pcb = pmain.tile([128, 1024], F32, tag="mm")
                pcs.append(pcb[:, 0:CW])
            for dd in range(2 * NG):
                for m in range(MQ):
                    nc.tensor.matmul(
                        pcs[m],
                        qqf[:, dd * 512 + m * 128 : dd * 512 + (m + 1) * 128],
                        kcf[:, dd * 128 : (dd + 1) * 128],
                        start=(dd == 0), stop=False,
                    )
            for m in range(MQ):
                nc.tensor.matmul(
                    pcs[m], qqb[:, m * 128 : (m + 1) * 128], kcb[:],
                    start=False, stop=True,
                )
                nc.vector.tensor_reduce(
                    out=anc[m][:, 4:5], in_=pcs[m], axis=AX.X, op=ALU.min
                )

            # ---------- main GEMM: two J-groups per [128,1024] PSUM tile ----------
            last = NJ // 2 - 1
            for pair in range(NJ // 2):
                for m in range(MQ):
                    pt = pmain.tile([128, 1024], F32, tag="mm")
                    for h in range(2):
                        J = 2 * pair + h
                        half = pt[:, h * 512 : (h + 1) * 512]
                        for g in range(NG):
                            rhs = kjfs[J][:, g * 1024 : (g + 1) * 1024].rearrange(
                                "p (i c) -> p i c", i=2
                            )
                            nc.tensor.matmul(
                                half, q_lhs(g, m), rhs, start=(g == 0), stop=False,
                                perf_mode=DR,
                            )
                        nc.tensor.matmul(
                            half, qqb[:, m * 128 : (m + 1) * 128], kjbs[J][:],
                            start=False, stop=True,
                        )
                    nc.vector.tensor_reduce(
                        out=anc[m][:, pair : pair + 1], in_=pt[:],
                        axis=AX.X, op=ALU.min,
                    )
                    # same-class window [m*128, m*128+256) in rotated key space
                    if pair == 0:
                        nc.vector.tensor_reduce(
                            out=apan[:, m : m + 1],
                            in_=pt[:, m * 128 : m * 128 + 256],
                            axis=AX.X, op=ALU.max,
                        )
                    if pair == last:
                        nc.vector.tensor_reduce(
                            out=apan[:, MQ + m : MQ + m + 1], in_=anc[m][:],
                            axis=AX.X, op=ALU.min,
                        )

            # ---------- epilogue: ship [apm|anmin] transposed (host finishes) ----------
            ptrb = pmain.tile([128, 1024], F32, tag="mm")
            ptr = ptrb[0 : 2 * MQ, 0:128]
            nc.tensor.transpose(ptr, apan[:], identf[:])
            outs = const.tile([2 * MQ, 128], F32)
            nc.vector.tensor_copy(outs[:], ptr)
            nc.sync.dma_start(out=out_h[:, :], in_=outs[:])

    nc.finalize()
    return nc


def _get_nc():
    global _nc_cache
    if _nc_cache is None:
        _nc_cache = _build()
    return _nc_cache


def _prep(inputs, targets, center):
    x = np.ascontiguousarray(np.asarray(inputs, dtype=np.float32))
    t = np.asarray(targets).astype(np.int64).ravel()
    c = np.ascontiguousarray(np.asarray(center, dtype=np.float32))
    assert x.shape == (N, D) and t.shape == (N,) and c.shape == (P, D)
    bf = ml_dtypes.bfloat16
    f8 = ml_dtypes.float8_e4m3

    order = np.argsort(t, kind="stable")
    xs = x[order]
    ts = t[order]
    _, counts = np.unique(ts, return_counts=True)
    assert counts.max() <= WIN + 1, f"class size {counts.max()} exceeds window {WIN + 1}"

    sq = np.sum(xs * xs, axis=1, dtype=np.float32)              # [N]
    sq_hi = sq.astype(bf)
    sq_lo = (sq - sq_hi.astype(np.float32)).astype(bf)

    cn = c / np.linalg.norm(c, axis=1, keepdims=True)           # f32 [P, D]
    csq = np.sum(cn * cn, axis=1, dtype=np.float32)             # [P]
    csq_hi = csq.astype(bf)
    csq_lo = (csq - csq_hi.astype(np.float32)).astype(bf)

    # class row layout: classes 0..95 -> rows 0..95; rows 96,97 = sq hi/lo;
    # classes 96..99 -> rows 98..101
    cls_row = np.where(ts < 96, ts, ts + 2).astype(np.int64)

    X8 = xs.T.astype(f8)                                        # [768, 4096] keys
    K6 = np.zeros((128, N), dtype=np.float32)
    K6[cls_row, np.arange(N)] = BIG
    K6[96, :] = sq_hi.astype(np.float32)
    K6[97, :] = sq_lo.astype(np.float32)
    K6 = K6.astype(bf)

    # centers: fp8 data rows + bf16 csq block (pad cols lose every min)
    cn8 = np.zeros((D, CW), dtype=f8)
    cn8[:, :P] = cn.T.astype(f8)
    kcf = np.ascontiguousarray(
        cn8.reshape(NG * 2, 128, CW).transpose(1, 0, 2)
    ).reshape(128, NG * 256)
    kcb = np.zeros((128, CW), dtype=np.float32)
    kcb[96, :P] = csq_hi.astype(np.float32)
    kcb[97, :P] = csq_lo.astype(np.float32)
    kcb[96, P:] = PAD_U
    kcb = np.ascontiguousarray(kcb.astype(bf))

    Q6 = np.zeros((128, N), dtype=np.float32)
    Q6[cls_row, np.arange(N)] = 1.0
    Q6[96, :] = 1.0
    Q6[97, :] = 1.0
    Q6 = Q6.astype(bf)
    Q8 = (-2.0 * xs.T).astype(f8)                               # [768, 4096]

    maps = []
    for core in range(N_CORES):
        s = slice(core * NQ, (core + 1) * NQ)
        roll = (core * NQ - WIN) % N
        perm = (np.arange(N) + roll) % N
        # kjf[J*128+p, g*1024+i*512+c] = X8rot[(2g+i)*128+p, J*512+c]
        kjf = np.ascontiguousarray(
            X8[:, perm].reshape(NG * 2, 128, NJ, 512).transpose(2, 1, 0, 3)
        ).reshape(NJ * 128, NG * 1024)
        # kjb[J*128+p, c] = K6rot[p, J*512+c]
        kjb = np.ascontiguousarray(
            K6[:, perm].reshape(128, NJ, 512).transpose(1, 0, 2)
        ).reshape(NJ * 128, 512)
        # qqf[p, g*1024+i*512+c] = Q8[(2g+i)*128+p, core cols]
        qqf = np.ascontiguousarray(
            Q8[:, s].reshape(NG * 2, 128, NQ).transpose(1, 0, 2)
        ).reshape(128, NG * 1024)
        qqb = np.ascontiguousarray(Q6[:, s])
        maps.append({
            "kjf": kjf, "kjb": kjb, "qqf": qqf, "qqb": qqb,
            "kcf": kcf, "kcb": kcb,
        })
    return maps, sq


def run(inputs, targets, center, trace=False):
    nc = _get_nc()
    maps, sq = _prep(inputs, targets, center)
    res = run_bass_kernel_spmd(nc, maps, list(range(N_CORES)), trace=trace)
    total = 0.0
    for core, r in enumerate(res.results):
        o = np.asarray(r["out"], dtype=np.float64)              # [2*MQ, 128]
        sqc = sq[core * NQ : (core + 1) * NQ].astype(np.float64)
        ap2 = o[:MQ].reshape(NQ) - BIG + sqc
        an2 = o[MQ:].reshape(NQ) + sqc
        ap = np.sqrt(np.maximum(ap2, 1e-12))
        an = np.sqrt(np.maximum(an2, 1e-12))
        total += np.maximum(MARGIN + ap - an, 0.0).sum()
    loss = np.float32(total / N)
    return np.asarray(loss), res


def kernel(inputs, targets, center):
    out, _ = run(inputs, targets, center)
    return out


# revision 38
# speedup vs baseline: 1.0620x; 1.0620x over previous
"""AugmentedTripletLoss Trainium2 kernel — 8-core SPMD, row-sharded.

Math (matches reference):
  d2[i,j] = sq_i + sq_j - 2*X@X.T
  ap_i    = sqrt(clip(max_{same class}(d2), 1e-12))
  an_i    = min over (diff-class keys  union  normalized centers) of dist
  loss    = mean(relu(1 + ap - an))

Device strategy (per core, 512 query rows):
  Host sorts rows by class and packs an augmented GEMM so that
  u[q,j] = -2*x_q.x_j + sq_j + BIG*[same class] lands directly in PSUM:
    * data rows (768) as fp8 e4m3, contracted with DoubleRow matmuls
      (two 128-row k-tiles per instruction, ~1.5x bf16 rate),
    * the class/sq block (BIG*onehot + sq hi/lo) as one bf16 k-tile.
  Per-row an = min_j u is one DVE min-reduce per [128,1024] PSUM tile
  (two 512-column key groups share a 2-bank tile; 4 such tiles rotate
  through all 8 PSUM banks). Because rows are class-sorted and each
  core's key columns are rotated by (core*512 - 64), every query tile's
  same-class columns fall in the static window [t*128, t*128+256) of
  the first pair-tile — the ap max-reduce touches one 256-wide slice
  per tile instead of every tile (requires max class size <= 65,
  asserted at prep time).
  Centers (no BIG, csq instead of sq) + padding (u=8192, loses every
  min) ride as a separate 128-column group, min-reduced only.
  Output: per-core [apmax|anmin] [8,128] f32, PE-transposed so the
  store is 8 fat descriptors; host adds sq, does sqrt/relu/mean
  (no device collective). fp8/bf16 split keeps rel err ~1e-4.
"""
import sys

for _p in ("/opt/trn_rl_repo", "/root/.axon_site"):
    if _p not in sys.path:
        sys.path.insert(0, _p)

import numpy as np
import ml_dtypes

import concourse.bass as bass
import concourse.bacc as bacc
import concourse.mybir as mybir
from concourse.tile import TileContext
from concourse.bass_utils import run_bass_kernel_spmd

F32 = mybir.dt.float32
BF16 = mybir.dt.bfloat16
FP8 = mybir.dt.float8e4
ALU = mybir.AluOpType
ACTF = mybir.ActivationFunctionType
AX = mybir.AxisListType
DR = mybir.MatmulPerfMode.DoubleRow

N_CORES = 8
N, D, P = 4096, 768, 100
NQ = N // N_CORES              # 512 query rows per core
MQ = NQ // 128                 # 4 query m-tiles
NG = 3                         # DoubleRow groups (6 fp8 k-tiles of 128)
NJ = 8                         # key column groups of 512
CW = 128                       # center group width (100 centers + 28 pad)
BIG = 16384.0
PAD_U = 8192.0
MARGIN = 1.0
WIN = 64                       # class half-window (max class size must be <=65)

_nc_cache = None


def _build():
    nc = bacc.Bacc("TRN2", target_bir_lowering=False, num_devices=N_CORES)

    kjf_h = nc.declare_dram_parameter("kjf", [NJ * 128, NG * 1024], FP8, isOutput=False)
    kjb_h = nc.declare_dram_parameter("kjb", [NJ * 128, 512], BF16, isOutput=False)
    qqf_h = nc.declare_dram_parameter("qqf", [128, NG * 1024], FP8, isOutput=False)
    qqb_h = nc.declare_dram_parameter("qqb", [128, 512], BF16, isOutput=False)
    kcf_h = nc.declare_dram_parameter("kcf", [128, NG * 256], FP8, isOutput=False)
    kcb_h = nc.declare_dram_parameter("kcb", [128, CW], BF16, isOutput=False)
    out_h = nc.declare_dram_parameter("out", [2 * MQ, 128], F32, isOutput=True)

    with TileContext(nc) as tc:
        from contextlib import ExitStack

        with ExitStack() as ctx:
            const = ctx.enter_context(tc.tile_pool(name="const", bufs=1))
            pmain = ctx.enter_context(tc.tile_pool(name="pmain", bufs=4, space="PSUM"))

            # ---------- loads (two HWDGE rings, issue order = need order) ----------
            qqf = const.tile([128, NG * 1024], FP8)
            kjfs = [
                const.tile([128, NG * 1024], FP8, name=f"kjf{J}") for J in range(NJ)
            ]
            kjbs = [const.tile([128, 512], BF16, name=f"kjb{J}") for J in range(NJ)]

            def load_qqf(g):
                nc.sync.dma_start(
                    out=qqf[:, g * 1024 : (g + 1) * 1024],
                    in_=qqf_h[:, g * 1024 : (g + 1) * 1024],
                )

            def load_kjf(J, g):
                nc.sync.dma_start(
                    out=kjfs[J][:, g * 1024 : (g + 1) * 1024],
                    in_=kjf_h[J * 128 : (J + 1) * 128, g * 1024 : (g + 1) * 1024],
                )

            load_qqf(0)
            load_qqf(1)
            load_qqf(2)
            load_kjf(0, 0)
            load_kjf(0, 1)
            load_kjf(0, 2)
            kcf = const.tile([128, NG * 256], FP8)
            nc.scalar.dma_start(out=kcf[:], in_=kcf_h[:, :])
            kcb = const.tile([128, CW], BF16)
            nc.scalar.dma_start(out=kcb[:], in_=kcb_h[:, :])
            qqb = const.tile([128, 512], BF16)
            nc.scalar.dma_start(out=qqb[:], in_=qqb_h[:, :])
            for J in range(NJ):
                if J > 0:
                    nc.sync.dma_start(
                        out=kjfs[J][:], in_=kjf_h[J * 128 : (J + 1) * 128, :]
                    )
                nc.scalar.dma_start(
                    out=kjbs[J][:], in_=kjb_h[J * 128 : (J + 1) * 128, :]
                )

            from concourse.masks import make_identity
            identf = const.tile([128, 128], F32)
            make_identity(nc, identf[:])

            apan = const.tile([128, 2 * MQ], F32)
            # per-m running-min columns: pairs 0..3 -> 0..3, centers -> 4
            anc = [const.tile([128, 5], F32, name=f"an{m}") for m in range(MQ)]

            def q_lhs(g, m):
                return qqf[:, g * 1024 : (g + 1) * 1024].rearrange(
                    "p (i c) -> p i c", i=2
                )[:, :, m * 128 : (m + 1) * 128]

            # ---------- centers first (overlaps kj DMA ramp) ----------
            for m in range(MQ):
                pcb = pmain.tile([128, 1024], F32, tag="mm")
                pc = pcb[:, 0:CW]
                for dd in range(2 * NG):
                    g, i = dd // 2, dd % 2
                    nc.tensor.matmul(
                        pc,
                        qqf[:, g * 1024 + i * 512 + m * 128
                            : g * 1024 + i * 512 + (m + 1) * 128],
                        kcf[:, dd * 128 : (dd + 1) * 128],
                        start=(dd == 0), stop=False,
                    )
                nc.tensor.matmul(
                    pc, qqb[:, m * 128 : (m + 1) * 128], kcb[:],
                    start=False, stop=True,
                )
                nc.vector.tensor_reduce(
                    out=anc[m][:, 4:5], in_=pc, axis=AX.X, op=ALU.min
                )

            # ---------- main GEMM: two J-groups per [128,1024] PSUM tile ----------
            last = NJ // 2 - 1
            for pair in range(NJ // 2):
                for m in range(MQ):
                    pt = pmain.tile([128, 1024], F32, tag="mm")
                    for h in range(2):
                        J = 2 * pair + h
                        half = pt[:, h * 512 : (h + 1) * 512]
                        for g in range(NG):
                            rhs = kjfs[J][:, g * 1024 : (g + 1) * 1024].rearrange(
                                "p (i c) -> p i c", i=2
                            )
                            nc.tensor.matmul(
                                half, q_lhs(g, m), rhs, start=(g == 0), stop=False,
                                perf_mode=DR,
                            )
                        nc.tensor.matmul(
                            half, qqb[:, m * 128 : (m + 1) * 128], kjbs[J][:],
                            start=False, stop=True,
                        )
                    nc.vector.tensor_reduce(
                        out=anc[m][:, pair : pair + 1], in_=pt[:],
                        axis=AX.X, op=ALU.min,
                    )
                    # same-class window [m*128, m*128+256) in rotated key space
                    if pair == 0:
                        nc.vector.tensor_reduce(
                            out=apan[:, m : m + 1],
                            in_=pt[:, m * 128 : m * 128 + 256],
                            axis=AX.X, op=ALU.max,
                        )
                    if pair == last:
                        nc.vector.tensor_reduce(
                            out=apan[:, MQ + m : MQ + m + 1], in_=anc[m][:],
                            axis=AX.X, op=ALU.min,
                        )

            # ---------- epilogue: ship [apm|anmin] transposed (host finishes) ----------
            ptrb = pmain.tile([128, 1024], F32, tag="mm")
            ptr = ptrb[0 : 2 * MQ, 0:128]
            nc.tensor.transpose(ptr, apan[:], identf[:])
            outs = const.tile([2 * MQ, 128], F32)
            nc.vector.tensor_copy(outs[:], ptr)
            nc.sync.dma_start(out=out_h[:, :], in_=outs[:])

    nc.finalize()
    return nc


def _get_nc():
    global _nc_cache
    if _nc_cache is None:
        _nc_cache = _build()
    return _nc_cache


def _prep(inputs, targets, center):
    x = np.ascontiguousarray(np.asarray(inputs, dtype=np.float32))
    t = np.asarray(targets).astype(np.int64).ravel()
    c = np.ascontiguousarray(np.asarray(center, dtype=np.float32))
    assert x.shape == (N, D) and t.shape == (N,) and c.shape == (P, D)
    bf = ml_dtypes.bfloat16
    f8 = ml_dtypes.float8_e4m3

    order = np.argsort(t, kind="stable")
    xs = x[order]
    ts = t[order]
    _, counts = np.unique(ts, return_counts=True)
    assert counts.max() <= WIN + 1, f"class size {counts.max()} exceeds window {WIN + 1}"

    sq = np.sum(xs * xs, axis=1, dtype=np.float32)              # [N]
    sq_hi = sq.astype(bf)
    sq_lo = (sq - sq_hi.astype(np.float32)).astype(bf)

    cn = c / np.linalg.norm(c, axis=1, keepdims=True)           # f32 [P, D]
    csq = np.sum(cn * cn, axis=1, dtype=np.float32)             # [P]
    csq_hi = csq.astype(bf)
    csq_lo = (csq - csq_hi.astype(np.float32)).astype(bf)

    # class row layout: classes 0..95 -> rows 0..95; rows 96,97 = sq hi/lo;
    # classes 96..99 -> rows 98..101
    cls_row = np.where(ts < 96, ts, ts + 2).astype(np.int64)

    X8 = xs.T.astype(f8)                                        # [768, 4096] keys
    K6 = np.zeros((128, N), dtype=np.float32)
    K6[cls_row, np.arange(N)] = BIG
    K6[96, :] = sq_hi.astype(np.float32)
    K6[97, :] = sq_lo.astype(np.float32)
    K6 = K6.astype(bf)

    # centers: fp8 data rows + bf16 csq block (pad cols lose every min)
    cn8 = np.zeros((D, CW), dtype=f8)
    cn8[:, :P] = cn.T.astype(f8)
    kcf = np.ascontiguousarray(
        cn8.reshape(NG * 2, 128, CW).transpose(1, 0, 2)
    ).reshape(128, NG * 256)
    kcb = np.zeros((128, CW), dtype=np.float32)
    kcb[96, :P] = csq_hi.astype(np.float32)
    kcb[97, :P] = csq_lo.astype(np.float32)
    kcb[96, P:] = PAD_U
    kcb = np.ascontiguousarray(kcb.astype(bf))

    Q6 = np.zeros((128, N), dtype=np.float32)
    Q6[cls_row, np.arange(N)] = 1.0
    Q6[96, :] = 1.0
    Q6[97, :] = 1.0
    Q6 = Q6.astype(bf)
    Q8 = (-2.0 * xs.T).astype(f8)                               # [768, 4096]

    maps = []
    for core in range(N_CORES):
        s = slice(core * NQ, (core + 1) * NQ)
        roll = (core * NQ - WIN) % N
        perm = (np.arange(N) + roll) % N
        # kjf[J*128+p, g*1024+i*512+c] = X8rot[(2g+i)*128+p, J*512+c]
        kjf = np.ascontiguousarray(
            X8[:, perm].reshape(NG * 2, 128, NJ, 512).transpose(2, 1, 0, 3)
        ).reshape(NJ * 128, NG * 1024)
        # kjb[J*128+p, c] = K6rot[p, J*512+c]
        kjb = np.ascontiguousarray(
            K6[:, perm].reshape(128, NJ, 512).transpose(1, 0, 2)
        ).reshape(NJ * 128, 512)
        # qqf[p, g*1024+i*512+c] = Q8[(2g+i)*128+p, core cols]
        qqf = np.ascontiguousarray(
            Q8[:, s].reshape(NG * 2, 128, NQ).transpose(1, 0, 2)
        ).reshape(128, NG * 1024)
        qqb = np.ascontiguousarray(Q6[:, s])
        maps.append({
            "kjf": kjf, "kjb": kjb, "qqf": qqf, "qqb": qqb,
            "kcf": kcf, "kcb": kcb,
        })
    return maps, sq


def run(inputs, targets, center, trace=False):
    nc = _get_nc()
    maps, sq = _prep(inputs, targets, center)
    res = run_bass_kernel_spmd(nc, maps, list(range(N_CORES)), trace=trace)
    total = 0.0
    for core, r in enumerate(res.results):
        o = np.asarray(r["out"], dtype=np.float64)              # [2*MQ, 128]
        sqc = sq[core * NQ : (core + 1) * NQ].astype(np.float64)
        ap2 = o[:MQ].reshape(NQ) - BIG + sqc
        an2 = o[MQ:].reshape(NQ) + sqc
        ap = np.sqrt(np.maximum(ap2, 1e-12))
        an = np.sqrt(np.maximum(an2, 1e-12))
        total += np.maximum(MARGIN + ap - an, 0.0).sum()
    loss = np.float32(total / N)
    return np.asarray(loss), res


def kernel(inputs, targets, center):
    out, _ = run(inputs, targets, center)
    return out
